# revision 1
# baseline (speedup 1.0000x reference)
"""Trainium2 Bass kernel for nn_Net_stacked_modified (dense_mlp, ridge).

Strategy: SINGLE NeuronCore. The model is a 50-step sequential scan where
every step applies two training-mode BatchNorms over the full batch (B=2048).
Any batch sharding needs ~100 sequential cross-core stat all-reduces; on this
runtime an 8-core ncfw AllReduce measures ~409us and the SWDGE remote-DMA
path does not compile ("ISA wrong length"), so multi-core designs are >40x
slower than one well-used core. Everything runs on core 0.

Layout: feature-major ("transposed") activations [feature_part, batch_free]
so BN statistics are free-axis reductions and BN apply is a per-partition
affine. The BN scale (g*rsqrt(var+eps) > 0) is folded into the rows of the
NEXT layer's weight matrix (relu(a*x) = a*relu(x) for a>0), so the apply is
a single add+max (shift+relu) pass. Matmuls run as float32r (full PE rate
for free dim >= 256). The value process v is accumulated entirely in PSUM
via +/-ones matmuls over per-step product tensors.
"""
import sys
import numpy as np

sys.path.insert(0, "/opt/trn_rl_repo")

import concourse.bass as bass  # noqa: E402
import concourse.bacc as bacc  # noqa: E402
import concourse.mybir as mybir  # noqa: E402
from concourse import tile  # noqa: E402
from concourse.bass_utils import run_bass_kernel_spmd  # noqa: E402

F32 = mybir.dt.float32
F32R = mybir.dt.float32r
BF16 = mybir.dt.bfloat16
AF = mybir.ActivationFunctionType
OP = mybir.AluOpType

KAPPA = 1.0
SIGMA = 0.3
EPS = 1e-5

_CACHE = {}


def _r(ap):
    return ap.bitcast(F32R)


def _build(S, B, D, H, hs):
    import os
    DBG = int(os.environ.get('DBG_STAGE', '0'))
    """Build the single-core Bass program. hs = python list of step sizes."""
    assert B == 2048 and D == 256 and H == 266
    NB = B // 512          # 512-wide matmul blocks
    KD = D // 128          # 2 k-tiles for D
    KH = 3                 # k-tiles for H (128,128,10->padded 128)
    CW = [128, 128, 10]    # H chunk widths
    import contextlib

    nc = bacc.Bacc(None, target_bir_lowering=False)
    dp = nc.declare_dram_parameter
    xt_d = dp("xt", [128, KD * B], F32, isOutput=False)
    dwt_d = dp("dwt", [S, 128, KD * B], F32, isOutput=False)
    w1_d = dp("w1p", [S, 128, KD * H], F32, isOutput=False)
    w2_d = dp("w2p", [S, 128, KH * H], F32, isOutput=False)
    w3_d = dp("w3p", [S, 128, KH * D], F32, isOutput=False)
    law_d = dp("law2", [128, S * KD], F32, isOutput=False)
    b3_d = dp("b3p", [128, S * KD], F32, isOutput=False)
    g1_d = dp("g1p", [128, S * 3], F32, isOutput=False)
    be1_d = dp("be1p", [128, S * 3], F32, isOutput=False)
    g2_d = dp("g2p", [128, S * 3], F32, isOutput=False)
    be2_d = dp("be2p", [128, S * 3], F32, isOutput=False)
    wv1_d = dp("wv1p", [128, KD * H], F32, isOutput=False)
    wv2_d = dp("wv2p", [128, KH * H], F32, isOutput=False)
    gv1_d = dp("gv1p", [128, 3], F32, isOutput=False)
    bev1_d = dp("bev1p", [128, 3], F32, isOutput=False)
    gv2_d = dp("gv2p", [128, 3], F32, isOutput=False)
    bev2_d = dp("bev2p", [128, 3], F32, isOutput=False)
    vout_d = dp("vout", [128, 1024], F32, isOutput=True)
    hv2_d = dp("hv2out", [128, 3 * B], F32, isOutput=True)
    av2_d = dp("av2out", [128, 3], F32, isOutput=True)

    ctx = contextlib.ExitStack()
    with ctx:
        sb = lambda name, shape, dt=F32: ctx.enter_context(nc.sbuf_tensor(name, shape, dt))
        ps = lambda name, shape: ctx.enter_context(nc.psum_tensor(name, shape, F32))

        xc = sb("xc", [128, KD * B])
        hA = sb("hA", [128, 3 * B])
        hB = sb("hB", [128, 3 * B])
        gsb = sb("gsb", [128, KD * B])
        dwt = [sb(f"dwt{i}", [128, KD * B]) for i in range(2)]
        w1 = [sb(f"w1_{i}", [128, KD * H]) for i in range(2)]
        w1r = [sb(f"w1r{i}", [128, KD * H], F32R) for i in range(2)]
        w2 = [sb(f"w2_{i}", [128, KH * H]) for i in range(2)]
        w3 = [sb(f"w3_{i}", [128, KH * D]) for i in range(2)]
        w2s = sb("w2s", [128, KH * H])
        w3s = sb("w3s", [128, KH * D])
        lawsb = sb("lawsb", [128, S * KD])
        b3sb = sb("b3sb", [128, S * KD])
        lawc = sb("lawc", [128, S * KD])
        b3c = sb("b3c", [128, S * KD])
        g1sb = sb("g1sb", [128, S * 3])
        be1sb = sb("be1sb", [128, S * 3])
        g2sb = sb("g2sb", [128, S * 3])
        be2sb = sb("be2sb", [128, S * 3])
        wv1sb = sb("wv1sb", [128, KD * H])
        wv1r = sb("wv1r", [128, KD * H], F32R)
        gv1sb = sb("gv1sb", [128, 3])
        bev1sb = sb("bev1sb", [128, 3])
        gv2sb = sb("gv2sb", [128, 3])
        bev2sb = sb("bev2sb", [128, 3])
        # stats scratch
        sy6 = sb("sy6", [128, 6])
        ss6 = sb("ss6", [128, 6])
        stt1 = sb("stt1", [128, 3])   # temp
        stt2 = sb("stt2", [128, 3])
        mu3 = sb("mu3", [128, 3])
        inv3 = sb("inv3", [128, 3])
        asc = sb("asc", [128, 3])     # a = g*inv
        shf = sb("shf", [128, 3])     # shift
        sqscr = sb("sqscr", [128, 1024])
        pb1 = sb("pb1", [128, KD * B], BF16)
        pb2 = sb("pb2", [128, KD * B], BF16)
        pb4 = sb("pb4", [128, KD * B], BF16)
        onesp = sb("onesp", [128, 1], BF16)
        onesn = sb("onesn", [128, 1], BF16)
        onesf = sb("onesf", [128, 1])
        vsb = sb("vsb", [128, 1024])

        yps = [ps(f"yps{i}", [128, 1024]) for i in range(2)]
        gps = ps("gps", [128, 1024])
        vps = ps("vps", [128, 1024])

        with tile.TileContext(nc) as tc:
            V, A, G_, T, SY = nc.vector, nc.scalar, nc.gpsimd, nc.tensor, nc.sync

            def dma(dst, src):
                SY.dma_start(out=dst, in_=src)

            # ---- one-time loads ----
            dma(gsb[:, :], xt_d[:, :])
            V.tensor_copy(_r(xc[:, :]), gsb[:, :])
            dma(lawsb[:, :], law_d[:, :])
            dma(b3sb[:, :], b3_d[:, :])
            for d_, s_ in ((g1_d, g1sb), (be1_d, be1sb), (g2_d, g2sb),
                           (be2_d, be2sb), (wv1_d, wv1sb),
                           (gv1_d, gv1sb), (bev1_d, bev1sb), (gv2_d, gv2sb),
                           (bev2_d, bev2sb)):
                dma(s_[:, :], d_[:, :])
            dma(w2[0][:, :], wv2_d[:, :])   # v0 L2 weights into w2 slot 0
            dma(dwt[0][:, :], dwt_d[0])
            dma(w1[0][:, :], w1_d[0])
            V.tensor_copy(lawc[:, :], lawsb[:, :])
            V.tensor_copy(b3c[:, :], b3sb[:, :])
            G_.memset(onesf[:, :], 1.0)
            V.tensor_copy(onesp[:, :], onesf[:, :])
            V.tensor_scalar_mul(onesn[:, :], onesf[:, :], -1.0)
            G_.memset(hA[:, :], 0.0)
            G_.memset(hB[:, :], 0.0)
            G_.memset(sy6[:, :], 0.0)
            G_.memset(ss6[:, :], 0.0)
            G_.memset(w2s[:, :], 0.0)
            G_.memset(vsb[:, :], 0.0)
            G_.memset(w3s[:, :], 0.0)

            def mm_layer(rhs_sb, w_sb, wofs, kt, fdim, nchunks, rhs_kstride):
                """y[c] = sum_k W[k, c-cols].T @ rhs[k] for all chunks; returns
                generator yielding (c, half, psum_tile) after its MMs are emitted."""
                i = 0
                for c in range(nchunks):
                    cw = CW[c] if nchunks == 3 else 128
                    for half in range(2):
                        pt = yps[i % 2]
                        for k in range(kt):
                            lhs = w_sb[:, wofs + k * fdim + c * 128:
                                       wofs + k * fdim + c * 128 + cw]
                            for sub in range(2):
                                col = half * 1024 + sub * 512
                                T.matmul(
                                    pt[0:cw, sub * 512:sub * 512 + 512],
                                    _r(lhs),
                                    _r(rhs_sb[:, k * rhs_kstride + col:
                                              k * rhs_kstride + col + 512]),
                                    start=(k == 0), stop=(k == kt - 1))
                        yield c, cw, half, pt
                        i += 1

            def bn_layer(rhs_sb, w_sb, wofs, kt, fdim, dst, g_ap, be_ap):
                """matmul + stats; returns nothing; fills dst with raw y and
                sy6/ss6 with per-(c,half) sums."""
                for c, cw, half, pt in mm_layer(rhs_sb, w_sb, wofs, kt, fdim, 3, B):
                    col6 = c * 2 + half
                    A.activation(_r(dst[0:cw, c * B + half * 1024:
                                        c * B + half * 1024 + 1024]),
                                 pt[0:cw, :], AF.Copy,
                                 accum_out=sy6[0:cw, col6:col6 + 1])
                    A.activation(sqscr[0:cw, :], pt[0:cw, :], AF.Square,
                                 accum_out=ss6[0:cw, col6:col6 + 1])

            def bn_finalize(g_ap, be_ap):
                """sy6/ss6 -> asc (a=g*inv), shf (shift=-mu+be/a)."""
                V.tensor_tensor(out=stt1[:, :], in0=sy6[:, 0:6:2],
                                in1=sy6[:, 1:6:2], op=OP.add)
                V.tensor_scalar_mul(mu3[:, :], stt1[:, :], 1.0 / B)
                V.tensor_tensor(out=stt1[:, :], in0=ss6[:, 0:6:2],
                                in1=ss6[:, 1:6:2], op=OP.add)
                V.tensor_scalar_mul(stt1[:, :], stt1[:, :], 1.0 / B)
                V.tensor_tensor(out=stt2[:, :], in0=mu3[:, :], in1=mu3[:, :],
                                op=OP.mult)
                V.tensor_tensor(out=stt1[:, :], in0=stt1[:, :], in1=stt2[:, :],
                                op=OP.subtract)
                V.tensor_scalar_add(stt1[:, :], stt1[:, :], EPS)
                A.activation(stt1[:, :], stt1[:, :], AF.Sqrt)
                V.reciprocal(inv3[:, :], stt1[:, :])
                V.tensor_tensor(out=asc[:, :], in0=g_ap, in1=inv3[:, :],
                                op=OP.mult)
                V.reciprocal(stt2[:, :], asc[:, :])
                V.tensor_tensor(out=stt2[:, :], in0=be_ap, in1=stt2[:, :],
                                op=OP.mult)
                V.tensor_tensor(out=shf[:, :], in0=stt2[:, :], in1=mu3[:, :],
                                op=OP.subtract)

            def bn_apply(dst):
                """in-place relu(y + shift) per chunk; split across engines."""
                for c in range(3):
                    cw = CW[c]
                    if c == 2:
                        A.activation(_r(dst[0:cw, c * B:(c + 1) * B]),
                                     dst[0:cw, c * B:(c + 1) * B], AF.Relu,
                                     bias=shf[0:cw, c:c + 1])
                    else:
                        eng = G_ if c == 0 else V
                        eng.tensor_scalar(
                            out=_r(dst[0:cw, c * B:(c + 1) * B]),
                            in0=dst[0:cw, c * B:(c + 1) * B],
                            scalar1=shf[0:cw, c:c + 1], scalar2=0.0,
                            op0=OP.add, op1=OP.max)

            def wscale(dst, src, extra=None):
                """dst[k-tile rows] = src * a[k] (* extra)."""
                fd = src.shape[1] // KH
                for k in range(KH):
                    cw = CW[k]
                    if extra is None:
                        V.tensor_scalar_mul(_r(dst[0:cw, k * fd:(k + 1) * fd]),
                                            src[0:cw, k * fd:(k + 1) * fd],
                                            asc[0:cw, k:k + 1])
                    else:
                        V.tensor_scalar(out=_r(dst[0:cw, k * fd:(k + 1) * fd]),
                                        in0=src[0:cw, k * fd:(k + 1) * fd],
                                        scalar1=asc[0:cw, k:k + 1],
                                        scalar2=extra, op0=OP.mult, op1=OP.mult)

            # ================= v0 network =================
            V.tensor_copy(wv1r[:, :], wv1sb[:, :])
            if DBG != 10:
                bn_layer(xc, wv1r, 0, KD, H, hA, None, None)
                if DBG != 11:
                    bn_finalize(gv1sb[:, :], bev1sb[:, :])
                    bn_apply(hA)
                    wscale(w2s, w2[0])
                    if DBG != 12:
                        bn_layer(hA, w2s, 0, KH, H, hB, None, None)
                        bn_finalize(gv2sb[:, :], bev2sb[:, :])
                        bn_apply(hB)
            # export hv2 (unscaled) + av2; host computes z, BN, relu
            dma(hv2_d[:, :], hB[:, :])
            dma(av2_d[:, :], asc[:, :])

            # ================= the scan =================
            first_v = {n: True for n in range(4)}
            for s in range(S if DBG in (0, 4) else 0):
                bf = s % 2
                h = float(hs[s])
                sqh = float(np.sqrt(h))
                # prefetch next step
                if s + 1 < S:
                    dma(dwt[1 - bf][:, :], dwt_d[s + 1])
                    dma(w1[1 - bf][:, :], w1_d[s + 1])
                dma(w2[bf][:, :], w2_d[s])
                dma(w3[bf][:, :], w3_d[s])

                # L1
                V.tensor_copy(w1r[bf][:, :], w1[bf][:, :])
                if DBG == 4 and s > 0:
                    continue
                bn_layer(xc, w1r[bf], 0, KD, H, hA, None, None)
                bn_finalize(g1sb[:, 3 * s:3 * s + 3], be1sb[:, 3 * s:3 * s + 3])
                bn_apply(hA)
                wscale(w2s, w2[bf])
                # L2
                bn_layer(hA, w2s, 0, KH, H, hB, None, None)
                bn_finalize(g2sb[:, 3 * s:3 * s + 3], be2sb[:, 3 * s:3 * s + 3])
                bn_apply(hB)
                wscale(w3s, w3[bf], extra=-h)
                sqk = float(KAPPA * np.sqrt(h / 2.0))

                # L3: G = -h*grad ; then products + v-MMs + xc update
                for dc in range(KD):
                    for half in range(2):
                        for k in range(KH):
                            lhs = w3s[:, k * D + dc * 128:k * D + dc * 128 + 128]
                            for sub in range(2):
                                col = half * 1024 + sub * 512
                                T.matmul(
                                    gps[0:128, sub * 512:sub * 512 + 512],
                                    _r(lhs),
                                    _r(hB[:, k * B + col:k * B + col + 512]),
                                    start=(k == 0), stop=(k == KH - 1))
                        gofs = dc * B + half * 1024
                        # evac with bias -h*b3 (V: psum + per-partition b3')
                        V.tensor_scalar_add(gsb[:, gofs:gofs + 1024], gps[:, :],
                                            b3c[:, KD * s + dc:KD * s + dc + 1])
                        # P4 before xc update (reads xc_s): xl = xc - law (V),
                        # then Square(sqk*xl) on ACT with float scale only
                        V.tensor_scalar(
                            out=sqscr[:, 0:1024], in0=xc[:, gofs:gofs + 1024],
                            scalar1=lawc[:, KD * s + dc:KD * s + dc + 1],
                            scalar2=None, op0=OP.subtract, op1=OP.bypass)
                        A.activation(pb4[:, gofs:gofs + 1024],
                                     sqscr[:, 0:1024], AF.Square, scale=sqk)
                        # xc += G (psum-source)
                        V.scalar_tensor_tensor(
                            out=_r(xc[:, gofs:gofs + 1024]), in0=gps[:, :],
                            scalar=1.0, in1=xc[:, gofs:gofs + 1024],
                            op0=OP.mult, op1=OP.add)
                # products (full dc rows at once)
                for dc in range(KD):
                    o = dc * B
                    V.scalar_tensor_tensor(
                        out=pb1[:, o:o + B], in0=gsb[:, o:o + B],
                        scalar=float(-SIGMA / np.sqrt(h)), in1=dwt[bf][:, o:o + B],
                        op0=OP.mult, op1=OP.mult)
                    V.scalar_tensor_tensor(
                        out=pb2[:, o:o + B], in0=gsb[:, o:o + B],
                        scalar=float(-1.0 / (2.0 * h)), in1=gsb[:, o:o + B],
                        op0=OP.mult, op1=OP.mult)
                    # xc += sqh*sigma*dW
                    V.scalar_tensor_tensor(
                        out=_r(xc[:, o:o + B]), in0=dwt[bf][:, o:o + B],
                        scalar=float(SIGMA * sqh), in1=xc[:, o:o + B],
                        op0=OP.mult, op1=OP.add)
                # v accumulation MMs
                for term, (pb, ones) in enumerate(
                        ((pb1, onesp), (pb2, onesp), (pb4, onesn))):
                    for dc in range(KD):
                        for n in range(NB):
                            r0, c0 = 32 * (n // 2), 512 * (n % 2)
                            T.matmul(
                                vps[r0:r0 + 1, c0:c0 + 512],
                                ones[:, :],
                                pb[:, dc * B + n * 512:dc * B + n * 512 + 512],
                                start=first_v[n], stop=False,
                                skip_group_check=True)
                            first_v[n] = False

            # final: evacuate the v accumulator rows; host adds v0
            V.tensor_copy(vsb[0:1, :], vps[0:1, :])
            V.tensor_copy(vsb[32:33, :], vps[32:33, :])
            dma(vout_d[:, :], vsb[:, :])

    nc.compile()
    return nc


def _pack(inputs):
    hs_ = np.diff(np.asarray(inputs["timegrid"], np.float64))
    S = inputs["dW"].shape[0]
    B, D = inputs["x"].shape
    H = inputs["W1"].shape[2]
    f = np.float32
    im = {}
    im["xt"] = np.ascontiguousarray(
        inputs["x"].T.reshape(2, 128, B).transpose(1, 0, 2).reshape(128, 2 * B), f)
    im["dwt"] = np.ascontiguousarray(
        inputs["dW"].reshape(S, B, 2, 128).transpose(0, 3, 2, 1).reshape(S, 128, 2 * B), f)
    im["w1p"] = np.ascontiguousarray(
        inputs["W1"].reshape(S, 2, 128, H).transpose(0, 2, 1, 3).reshape(S, 128, 2 * H), f)

    def padk(w, fd):  # [S?, 266, fd] -> [S?, 128, 3*fd]
        w3 = np.zeros((w.shape[0], 3, 128, fd), f)
        w3[:, 0] = w[:, :128]
        w3[:, 1] = w[:, 128:256]
        w3[:, 2, :10] = w[:, 256:266]
        return np.ascontiguousarray(w3.transpose(0, 2, 1, 3).reshape(w.shape[0], 128, 3 * fd))

    im["w2p"] = padk(inputs["W2"], H)
    im["w3p"] = padk(inputs["W3"], D)
    im["law2"] = np.ascontiguousarray(
        inputs["law"].reshape(S, 2, 128).transpose(2, 0, 1).reshape(128, S * 2), f)
    im["b3p"] = np.ascontiguousarray(
        (-hs_[:, None] * inputs["b3"].astype(np.float64)).astype(f)
        .reshape(S, 2, 128).transpose(2, 0, 1).reshape(128, S * 2))

    def pad3(v, fill=0.0):  # [S, 266] -> [128, S*3]
        z = np.full((v.shape[0], 3, 128), fill, f)
        z[:, 0] = v[:, :128]
        z[:, 1] = v[:, 128:256]
        z[:, 2, :10] = v[:, 256:266]
        return np.ascontiguousarray(z.transpose(2, 0, 1).reshape(128, v.shape[0] * 3))

    im["g1p"] = pad3(inputs["g1"], 1.0)
    im["be1p"] = pad3(inputs["be1"])
    im["g2p"] = pad3(inputs["g2"], 1.0)
    im["be2p"] = pad3(inputs["be2"])
    im["wv1p"] = np.ascontiguousarray(
        inputs["Wv1"].reshape(2, 128, H).transpose(1, 0, 2).reshape(128, 2 * H), f)
    im["wv2p"] = padk(inputs["Wv2"][None], H)[0]

    def pad1(v, fill=0.0):  # [266] -> [128, 3]
        z = np.full((3, 128), fill, f)
        z[0] = v[:128]
        z[1] = v[128:256]
        z[2, :10] = v[256:266]
        return np.ascontiguousarray(z.T)

    im["gv1p"] = pad1(inputs["gv1"], 1.0)
    im["bev1p"] = pad1(inputs["bev1"])
    im["gv2p"] = pad1(inputs["gv2"], 1.0)
    im["bev2p"] = pad1(inputs["bev2"])
    return im


def kernel(**inputs):
    inputs = {k: np.asarray(v, np.float32) for k, v in inputs.items()}
    S = inputs["dW"].shape[0]
    B, D = inputs["x"].shape
    H = inputs["W1"].shape[2]
    hs = tuple(np.diff(inputs["timegrid"]).astype(np.float64).tolist())
    key = (S, B, D, H, hs)
    if key not in _CACHE:
        _CACHE[key] = _build(S, B, D, H, hs)
    nc = _CACHE[key]
    im = _pack(inputs)
    res = run_bass_kernel_spmd(nc, [im], [0])
    vout = res.results[0]["vout"]
    hv2 = res.results[0]["hv2out"]
    av2 = res.results[0]["av2out"]
    v_scan = np.concatenate([vout[0, 0:512], vout[0, 512:1024],
                             vout[32, 0:512], vout[32, 512:1024]])
    # reassemble hv2 [266, B]; z = (av2*Wv3)^T hv2
    hv2f = np.concatenate([hv2[:, 0:B], hv2[:, B:2 * B],
                           hv2[0:10, 2 * B:3 * B]], axis=0).astype(np.float64)
    a2f = np.concatenate([av2[:, 0], av2[:, 1], av2[0:10, 2]]).astype(np.float64)
    wv3 = np.asarray(inputs["Wv3"], np.float64).reshape(-1)
    z = (a2f * wv3) @ hv2f
    z = z + float(np.asarray(inputs["bv3"]).reshape(-1)[0])
    mu = z.mean()
    var = ((z - mu) ** 2).mean()
    gv3 = float(np.asarray(inputs["gv3"]).reshape(-1)[0])
    bev3 = float(np.asarray(inputs["bev3"]).reshape(-1)[0])
    v0 = np.maximum(gv3 * (z - mu) / np.sqrt(var + EPS) + bev3, 0.0)
    v = (v_scan.astype(np.float64) + v0).astype(np.float32)
    return v.reshape(B, 1)


if __name__ == "__main__":
    pass

